# revision 1
# baseline (speedup 1.0000x reference)
"""Direct Conv2d (full cross-correlation, pad=K-1) as a Bass/Tile kernel on 8
Trainium2 NeuronCores.

Problem: inp [32,128,56,60] f32 (ints 0..3), weight [256,128,3,3] f32 (ints
0..2), out [32,256,58,62] f32 = conv_general_dilated(pad=2, NCHW/OIHW).

Strategy:
- Data-parallel over batch: 4 images per core, weights replicated.
- bf16 operands are exact here (inputs are tiny integers; PE accumulates in
  fp32; max partial sum 128*9*3*2 = 6912 << 2^24), so the matmuls are
  bit-exact vs the f32 reference.
- Direct conv as 9 shifted matmuls accumulating in PSUM: contraction over
  C_IN=128 (partition dim), stationary lhsT = weight tap [ci,co_half],
  moving rhs = a flat contiguous window of the zero-padded input.
- Input is zero-padded in SBUF to [61 rows, 64 cols] per image so that every
  tap's rhs is a single contiguous 512-element window and the PSUM tile is a
  full contiguous bank [128, 8*64]. Columns x>=62 of each PSUM row block are
  garbage (wrap-around reads) and are simply never copied out.
"""

import os
from contextlib import ExitStack

import numpy as np
import ml_dtypes

import concourse.bass as bass
import concourse.mybir as mybir
import concourse.tile as tile
from concourse import bacc, bass_utils

# Problem shape (hardcoded per contract)
B, C_IN, C_OUT, K, H, W = 32, 128, 256, 3, 56, 60
HO, WO = H + K - 1, W + K - 1  # 58, 62
N_CORES = 8
BPC = B // N_CORES  # images per core
PY, PX = 61, 64  # padded input plane in SBUF (61 rows so last block reads stay in-bounds)
# Output row blocks: 7 blocks of 8 rows + 1 block of 2 rows (8*64=512 = one PSUM bank)
BLOCKS = [(y0, min(8, HO - y0)) for y0 in range(0, HO, 8)]

_CACHE = {}
LAST_RESULT = None  # test harness introspection


def _build_program():
    nc = bacc.Bacc(
        "TRN2", target_bir_lowering=False, debug=False, num_devices=N_CORES
    )
    bf16 = mybir.dt.bfloat16
    f32 = mybir.dt.float32

    x = nc.dram_tensor("x", [BPC, C_IN, H, W], bf16, kind="ExternalInput").ap()
    w = nc.dram_tensor("w", [K * K, C_IN, C_OUT], bf16, kind="ExternalInput").ap()
    y = nc.dram_tensor("y", [BPC, C_OUT, HO, WO], f32, kind="ExternalOutput").ap()

    with tile.TileContext(nc) as tc:
        with ExitStack() as ctx:
            const_pool = ctx.enter_context(tc.tile_pool(name="const", bufs=1))
            psum_pool = ctx.enter_context(
                tc.tile_pool(name="psum", bufs=8, space="PSUM")
            )
            out_pool = ctx.enter_context(tc.tile_pool(name="outs", bufs=8))

            in_pad = const_pool.tile([C_IN, BPC, PY, PX], bf16, tag="in_pad")
            w_sb = const_pool.tile([C_IN, K * K, C_OUT], bf16, tag="w_sb")

            nc.vector.memset(in_pad[:], 0.0)
            nc.sync.dma_start(w_sb[:], w.rearrange("t c o -> c t o"))
            for b in range(BPC):
                nc.sync.dma_start(in_pad[:, b, 2 : 2 + H, 2 : 2 + W], x[b])

            in_flat = in_pad.rearrange("p b y x -> p b (y x)")

            for b in range(BPC):
                for g in range(C_OUT // 128):
                    psum_ts = [
                        psum_pool.tile([128, 512], f32, tag="ps", name=f"ps_{b}_{g}_{i}")
                        for i in range(len(BLOCKS))
                    ]
                    for t in range(K * K):
                        kh, kw = divmod(t, K)
                        lhsT = w_sb[:, t, g * 128 : (g + 1) * 128]
                        for blk, (y0, r) in enumerate(BLOCKS):
                            s = (y0 + kh) * PX + kw
                            nc.tensor.matmul(
                                psum_ts[blk][:, : r * PX],
                                lhsT,
                                in_flat[:, b, s : s + r * PX],
                                start=(t == 0),
                                stop=(t == K * K - 1),
                            )
                    for blk, (y0, r) in enumerate(BLOCKS):
                        o = out_pool.tile([128, 8 * WO], f32, tag="o")
                        src = psum_ts[blk].rearrange("p (y x) -> p y x", x=PX)
                        dst = o.rearrange("p (y x) -> p y x", x=WO)
                        nc.vector.tensor_copy(dst[:, :r, :], src[:, :r, :WO])
                        nc.sync.dma_start(
                            y[b, g * 128 : (g + 1) * 128, y0 : y0 + r, :],
                            dst[:, :r, :],
                        )

    nc.compile()
    return nc


def kernel(inp: np.ndarray, weight: np.ndarray) -> np.ndarray:
    global LAST_RESULT
    if "nc" not in _CACHE:
        _CACHE["nc"] = _build_program()
    nc = _CACHE["nc"]

    inp_bf = np.ascontiguousarray(inp).astype(ml_dtypes.bfloat16)
    # [co, ci, kh, kw] -> [kh*kw, ci, co]
    w_t = np.ascontiguousarray(
        weight.transpose(2, 3, 1, 0).reshape(K * K, C_IN, C_OUT)
    ).astype(ml_dtypes.bfloat16)

    in_maps = [
        {"x": inp_bf[c * BPC : (c + 1) * BPC], "w": w_t} for c in range(N_CORES)
    ]
    res = bass_utils.run_bass_kernel_spmd(nc, in_maps, core_ids=list(range(N_CORES)))
    LAST_RESULT = res
    out = np.concatenate([res.results[c]["y"] for c in range(N_CORES)], axis=0)
    return out


# revision 3
# speedup vs baseline: 1.1055x; 1.1055x over previous
"""Direct Conv2d (full cross-correlation, pad=K-1) as a Bass/Tile kernel on 8
Trainium2 NeuronCores.

Problem: inp [32,128,56,60] f32 (ints 0..3), weight [256,128,3,3] f32 (ints
0..2), out [32,256,58,62] f32 = conv_general_dilated(pad=2, NCHW/OIHW).

Strategy:
- Data-parallel over batch: 4 images per core, weights replicated.
- bf16 operands are exact here (inputs are tiny integers; PE accumulates in
  fp32; max partial sum 128*9*3*2 = 6912 << 2^24), so the matmuls are
  bit-exact vs the f32 reference.
- Direct conv as 9 shifted matmuls accumulating in PSUM: contraction over
  C_IN=128 (partition dim), stationary lhsT = weight tap [ci,co_half],
  moving rhs = a flat contiguous window of the zero-padded input.
- Input is zero-padded in SBUF to [61 rows, 64 cols] per image so that every
  tap's rhs is a single contiguous 512-element window and the PSUM tile is a
  full contiguous bank [128, 8*64]. Columns x>=62 of each PSUM row block are
  garbage (wrap-around reads) and are simply never copied out.
"""

import os
from contextlib import ExitStack

import numpy as np
import ml_dtypes

import concourse.bass as bass
import concourse.mybir as mybir
import concourse.tile as tile
from concourse import bacc, bass_utils

# Problem shape (hardcoded per contract)
B, C_IN, C_OUT, K, H, W = 32, 128, 256, 3, 56, 60
HO, WO = H + K - 1, W + K - 1  # 58, 62
N_CORES = 8
BPC = B // N_CORES  # images per core
PY, PX = 61, 64  # padded input plane in SBUF (61 rows so last block reads stay in-bounds)
# Output row blocks: 7 blocks of 8 rows + 1 block of 2 rows (8*64=512 = one PSUM bank)
BLOCKS = [(y0, min(8, HO - y0)) for y0 in range(0, HO, 8)]

_CACHE = {}
LAST_RESULT = None  # test harness introspection


def _build_program():
    nc = bacc.Bacc(
        "TRN2", target_bir_lowering=False, debug=False, num_devices=N_CORES
    )
    bf16 = mybir.dt.bfloat16
    f32 = mybir.dt.float32

    x = nc.dram_tensor("x", [BPC, C_IN, H, W], bf16, kind="ExternalInput").ap()
    w = nc.dram_tensor("w", [K * K, C_IN, C_OUT], bf16, kind="ExternalInput").ap()
    y = nc.dram_tensor("y", [BPC, C_OUT, HO, WO], f32, kind="ExternalOutput").ap()

    with tile.TileContext(nc) as tc:
        with ExitStack() as ctx:
            const_pool = ctx.enter_context(tc.tile_pool(name="const", bufs=1))
            psum_pool = ctx.enter_context(
                tc.tile_pool(name="psum", bufs=8, space="PSUM")
            )
            out_pool = ctx.enter_context(tc.tile_pool(name="outs", bufs=8))

            w_sb = const_pool.tile([C_IN, K * K, C_OUT], bf16, tag="w_sb")
            nc.sync.dma_start(w_sb[:], w.rearrange("t c o -> c t o"))

            # One padded-input tile per image so Tile's dependency tracking
            # lets image-0 matmuls start as soon as image 0 is resident.
            in_pads = []
            for b in range(BPC):
                t = const_pool.tile([C_IN, PY, PX], bf16, tag=f"in_pad{b}")
                # zero only the padding strips, not the whole plane
                nc.gpsimd.memset(t[:, 0:2, :], 0.0)
                nc.gpsimd.memset(t[:, 2 + H :, :], 0.0)
                nc.gpsimd.memset(t[:, 2 : 2 + H, 0:2], 0.0)
                nc.gpsimd.memset(t[:, 2 : 2 + H, 2 + W :], 0.0)
                nc.sync.dma_start(t[:, 2 : 2 + H, 2 : 2 + W], x[b])
                in_pads.append(t.rearrange("p y x -> p (y x)"))

            for b in range(BPC):
                for g in range(C_OUT // 128):
                    psum_ts = [
                        psum_pool.tile([128, 512], f32, tag="ps", name=f"ps_{b}_{g}_{i}")
                        for i in range(len(BLOCKS))
                    ]
                    for t in range(K * K):
                        kh, kw = divmod(t, K)
                        lhsT = w_sb[:, t, g * 128 : (g + 1) * 128]
                        for blk, (y0, r) in enumerate(BLOCKS):
                            s = (y0 + kh) * PX + kw
                            nc.tensor.matmul(
                                psum_ts[blk][:, : r * PX],
                                lhsT,
                                in_pads[b][:, s : s + r * PX],
                                start=(t == 0),
                                stop=(t == K * K - 1),
                            )
                    for blk, (y0, r) in enumerate(BLOCKS):
                        o = out_pool.tile([128, 8 * WO], f32, tag="o")
                        src = psum_ts[blk].rearrange("p (y x) -> p y x", x=PX)
                        dst = o.rearrange("p (y x) -> p y x", x=WO)
                        nc.vector.tensor_copy(dst[:, :r, :], src[:, :r, :WO])
                        nc.sync.dma_start(
                            y[b, g * 128 : (g + 1) * 128, y0 : y0 + r, :],
                            dst[:, :r, :],
                        )

    nc.compile()
    return nc


def kernel(inp: np.ndarray, weight: np.ndarray) -> np.ndarray:
    global LAST_RESULT
    if "nc" not in _CACHE:
        _CACHE["nc"] = _build_program()
    nc = _CACHE["nc"]

    inp_bf = np.ascontiguousarray(inp).astype(ml_dtypes.bfloat16)
    # [co, ci, kh, kw] -> [kh*kw, ci, co]
    w_t = np.ascontiguousarray(
        weight.transpose(2, 3, 1, 0).reshape(K * K, C_IN, C_OUT)
    ).astype(ml_dtypes.bfloat16)

    in_maps = [
        {"x": inp_bf[c * BPC : (c + 1) * BPC], "w": w_t} for c in range(N_CORES)
    ]
    res = bass_utils.run_bass_kernel_spmd(nc, in_maps, core_ids=list(range(N_CORES)))
    LAST_RESULT = res
    out = np.concatenate([res.results[c]["y"] for c in range(N_CORES)], axis=0)
    return out


# revision 4
# speedup vs baseline: 1.5070x; 1.3632x over previous
"""Direct Conv2d (full cross-correlation, pad=K-1) as a Bass/Tile kernel on 8
Trainium2 NeuronCores.

Problem: inp [32,128,56,60] f32 (ints 0..3), weight [256,128,3,3] f32 (ints
0..2), out [32,256,58,62] f32 = conv_general_dilated(pad=2, NCHW/OIHW).

Strategy:
- Data-parallel over batch: 4 images per core, weights replicated.
- Inputs are tiny integers, so fp8e4m3 operands are exact (PE accumulates in
  fp32; max partial sum 128*9*3*2 = 6912 << 2^24). The matmuls are bit-exact
  vs the f32 reference.
- Direct conv as shifted matmuls accumulating in PSUM: contraction over
  C_IN=128 (partition dim), stationary lhsT = weight tap(s) [ci,co_half],
  moving rhs = flat contiguous window(s) of the zero-padded input.
- fp8 DoubleRow contracts TWO taps per matmul: taps are paired along kh
  (rhs windows row-shifted by one padded row = 64 elements, satisfying the
  16B-multiple stride rule), kh=2 taps are paired with an all-zero weight
  tap. 6 DoubleRow matmuls replace 9 plain ones per PSUM tile.
- Input is zero-padded in SBUF to [62 rows, 64 cols] per image so every rhs
  is a contiguous window and each PSUM tile is a full contiguous bank
  [128, 8*64]. Columns x>=62 of each PSUM row block are garbage (wrap-around
  reads) and are never copied out.
"""

import os
from contextlib import ExitStack

import numpy as np
import ml_dtypes

import concourse.bass as bass
import concourse.mybir as mybir
import concourse.tile as tile
from concourse import bacc, bass_utils

# Problem shape (hardcoded per contract)
B, C_IN, C_OUT, K, H, W = 32, 128, 256, 3, 56, 60
HO, WO = H + K - 1, W + K - 1  # 58, 62
N_CORES = 8
BPC = B // N_CORES  # images per core
PY, PX = 62, 64  # zero-padded input plane in SBUF
# Output row blocks: 7 blocks of 8 rows + 1 block of 2 rows (8*64=512 = one PSUM bank)
BLOCKS = [(y0, min(8, HO - y0)) for y0 in range(0, HO, 8)]

USE_FP8 = os.environ.get("KERNEL_BF16") != "1"

# DoubleRow tap pairing: slot order in the weight tensor. Pairs are adjacent
# (2p, 2p+1); kh=2 taps pair with a zero tap (None). rhs window for pair p
# starts at row y0+KH_BASE[p], col KW[p], second window one row below.
PAIR_TAPS = [
    ((0, 0), (1, 0)),
    ((0, 1), (1, 1)),
    ((0, 2), (1, 2)),
    ((2, 0), None),
    ((2, 1), None),
    ((2, 2), None),
]
N_SLOTS = 2 * len(PAIR_TAPS)

_CACHE = {}
LAST_RESULT = None  # test harness introspection


def _build_fp8():
    nc = bacc.Bacc("TRN2", target_bir_lowering=False, debug=False, num_devices=N_CORES)
    fp8 = mybir.dt.float8e4
    f32 = mybir.dt.float32

    x = nc.dram_tensor("x", [BPC, C_IN, H, W], fp8, kind="ExternalInput").ap()
    w = nc.dram_tensor("w", [N_SLOTS, C_IN, C_OUT], fp8, kind="ExternalInput").ap()
    y = nc.dram_tensor("y", [BPC, C_OUT, HO, WO], f32, kind="ExternalOutput").ap()

    with tile.TileContext(nc) as tc:
        with ExitStack() as ctx:
            const_pool = ctx.enter_context(tc.tile_pool(name="const", bufs=1))
            psum_pool = ctx.enter_context(tc.tile_pool(name="psum", bufs=8, space="PSUM"))
            out_pool = ctx.enter_context(tc.tile_pool(name="outs", bufs=8))

            w_sb = const_pool.tile([C_IN, N_SLOTS, C_OUT], fp8, tag="w_sb")
            nc.sync.dma_start(w_sb[:], w.rearrange("t c o -> c t o"))

            # One padded-input tile per image so Tile's dependency tracking
            # lets image-0 matmuls start as soon as image 0 is resident.
            in_pads = []
            for b in range(BPC):
                t = const_pool.tile([C_IN, PY, PX], fp8, tag=f"in_pad{b}")
                nc.gpsimd.memset(t[:, 0:2, :], 0.0)
                nc.gpsimd.memset(t[:, 2 + H :, :], 0.0)
                nc.gpsimd.memset(t[:, 2 : 2 + H, 0:2], 0.0)
                nc.gpsimd.memset(t[:, 2 : 2 + H, 2 + W :], 0.0)
                nc.sync.dma_start(t[:, 2 : 2 + H, 2 : 2 + W], x[b])
                in_pads.append(t.rearrange("p y x -> p (y x)"))

            npairs = len(PAIR_TAPS)
            for b in range(BPC):
                for g in range(C_OUT // 128):
                    psum_ts = [
                        psum_pool.tile([128, 512], f32, tag="ps", name=f"ps_{b}_{g}_{i}")
                        for i in range(len(BLOCKS))
                    ]
                    for p, (tap0, _) in enumerate(PAIR_TAPS):
                        kh0, kw = tap0
                        lhsT = w_sb[:, 2 * p : 2 * p + 2, g * 128 : (g + 1) * 128]
                        for blk, (y0, r) in enumerate(BLOCKS):
                            s = (y0 + kh0) * PX + kw
                            base = in_pads[b]
                            rhs = bass.AP(
                                base.tensor,
                                base.offset + s,
                                [list(base.ap)[0], [PX, 2], [1, r * PX]],
                            )
                            nc.tensor.matmul(
                                psum_ts[blk][:, : r * PX],
                                lhsT,
                                rhs,
                                start=(p == 0),
                                stop=(p == npairs - 1),
                                perf_mode=mybir.MatmulPerfMode.DoubleRow,
                            )
                    for blk, (y0, r) in enumerate(BLOCKS):
                        o = out_pool.tile([128, 8 * WO], f32, tag="o")
                        src = psum_ts[blk].rearrange("p (y x) -> p y x", x=PX)
                        dst = o.rearrange("p (y x) -> p y x", x=WO)
                        nc.vector.tensor_copy(dst[:, :r, :], src[:, :r, :WO])
                        nc.sync.dma_start(
                            y[b, g * 128 : (g + 1) * 128, y0 : y0 + r, :],
                            dst[:, :r, :],
                        )

    nc.compile()
    return nc


def _build_bf16():
    nc = bacc.Bacc("TRN2", target_bir_lowering=False, debug=False, num_devices=N_CORES)
    bf16 = mybir.dt.bfloat16
    f32 = mybir.dt.float32

    x = nc.dram_tensor("x", [BPC, C_IN, H, W], bf16, kind="ExternalInput").ap()
    w = nc.dram_tensor("w", [K * K, C_IN, C_OUT], bf16, kind="ExternalInput").ap()
    y = nc.dram_tensor("y", [BPC, C_OUT, HO, WO], f32, kind="ExternalOutput").ap()

    with tile.TileContext(nc) as tc:
        with ExitStack() as ctx:
            const_pool = ctx.enter_context(tc.tile_pool(name="const", bufs=1))
            psum_pool = ctx.enter_context(tc.tile_pool(name="psum", bufs=8, space="PSUM"))
            out_pool = ctx.enter_context(tc.tile_pool(name="outs", bufs=8))

            w_sb = const_pool.tile([C_IN, K * K, C_OUT], bf16, tag="w_sb")
            nc.sync.dma_start(w_sb[:], w.rearrange("t c o -> c t o"))

            in_pads = []
            for b in range(BPC):
                t = const_pool.tile([C_IN, PY, PX], bf16, tag=f"in_pad{b}")
                nc.gpsimd.memset(t[:, 0:2, :], 0.0)
                nc.gpsimd.memset(t[:, 2 + H :, :], 0.0)
                nc.gpsimd.memset(t[:, 2 : 2 + H, 0:2], 0.0)
                nc.gpsimd.memset(t[:, 2 : 2 + H, 2 + W :], 0.0)
                nc.sync.dma_start(t[:, 2 : 2 + H, 2 : 2 + W], x[b])
                in_pads.append(t.rearrange("p y x -> p (y x)"))

            for b in range(BPC):
                for g in range(C_OUT // 128):
                    psum_ts = [
                        psum_pool.tile([128, 512], f32, tag="ps", name=f"ps_{b}_{g}_{i}")
                        for i in range(len(BLOCKS))
                    ]
                    for t in range(K * K):
                        kh, kw = divmod(t, K)
                        lhsT = w_sb[:, t, g * 128 : (g + 1) * 128]
                        for blk, (y0, r) in enumerate(BLOCKS):
                            s = (y0 + kh) * PX + kw
                            nc.tensor.matmul(
                                psum_ts[blk][:, : r * PX],
                                lhsT,
                                in_pads[b][:, s : s + r * PX],
                                start=(t == 0),
                                stop=(t == K * K - 1),
                            )
                    for blk, (y0, r) in enumerate(BLOCKS):
                        o = out_pool.tile([128, 8 * WO], f32, tag="o")
                        src = psum_ts[blk].rearrange("p (y x) -> p y x", x=PX)
                        dst = o.rearrange("p (y x) -> p y x", x=WO)
                        nc.vector.tensor_copy(dst[:, :r, :], src[:, :r, :WO])
                        nc.sync.dma_start(
                            y[b, g * 128 : (g + 1) * 128, y0 : y0 + r, :],
                            dst[:, :r, :],
                        )

    nc.compile()
    return nc


def kernel(inp: np.ndarray, weight: np.ndarray) -> np.ndarray:
    global LAST_RESULT
    if "nc" not in _CACHE:
        _CACHE["nc"] = _build_fp8() if USE_FP8 else _build_bf16()
    nc = _CACHE["nc"]

    if USE_FP8:
        dt = ml_dtypes.float8_e4m3
        inp_c = np.ascontiguousarray(inp).astype(dt)
        # weight [co, ci, kh, kw] -> slots [N_SLOTS, ci, co]
        w_t = np.zeros((N_SLOTS, C_IN, C_OUT), dtype=dt)
        wt = weight.transpose(2, 3, 1, 0)  # [kh, kw, ci, co]
        for p, (tap0, tap1) in enumerate(PAIR_TAPS):
            w_t[2 * p] = wt[tap0[0], tap0[1]].astype(dt)
            if tap1 is not None:
                w_t[2 * p + 1] = wt[tap1[0], tap1[1]].astype(dt)
    else:
        dt = ml_dtypes.bfloat16
        inp_c = np.ascontiguousarray(inp).astype(dt)
        w_t = np.ascontiguousarray(
            weight.transpose(2, 3, 1, 0).reshape(K * K, C_IN, C_OUT)
        ).astype(dt)

    in_maps = [
        {"x": inp_c[c * BPC : (c + 1) * BPC], "w": w_t} for c in range(N_CORES)
    ]
    res = bass_utils.run_bass_kernel_spmd(nc, in_maps, core_ids=list(range(N_CORES)))
    LAST_RESULT = res
    out = np.concatenate([res.results[c]["y"] for c in range(N_CORES)], axis=0)
    return out


# revision 5
# speedup vs baseline: 1.8322x; 1.2158x over previous
"""Direct Conv2d (full cross-correlation, pad=K-1) as a Bass/Tile kernel on 8
Trainium2 NeuronCores.

Problem: inp [32,128,56,60] f32 (ints 0..3), weight [256,128,3,3] f32 (ints
0..2), out [32,256,58,62] f32 = conv_general_dilated(pad=2, NCHW/OIHW).

Strategy:
- Data-parallel over batch: 4 images per core, weights replicated.
- Inputs are tiny integers, so fp8e4m3 operands are exact (PE accumulates in
  fp32; max partial sum 128*9*3*2 = 6912 << 2^24). The matmuls are bit-exact
  vs the f32 reference.
- Direct conv as shifted matmuls accumulating in PSUM: contraction over
  C_IN=128 (partition dim), stationary lhsT = weight tap pair [ci,2,co_half],
  moving rhs = two flat windows of the zero-padded input.
- fp8 DoubleRow contracts TWO taps per matmul. Taps are paired so each pair's
  two rhs windows are a fixed element-stride apart in the padded plane; the
  odd 9th tap pairs with an all-zero weight tap.
- The input is zero-padded HOST-side to [62 rows, 64 cols] per image so the
  input DMAs are fully contiguous and no on-device memset is needed. Every
  rhs is a contiguous window and each PSUM tile is a full bank [128, 8*64].
  Columns x>=62 of each PSUM row block are garbage (wrap-around reads) and
  are never copied out.
"""

import os
from contextlib import ExitStack

import numpy as np
import ml_dtypes

import concourse.bass as bass
import concourse.mybir as mybir
import concourse.tile as tile
from concourse import bacc, bass_utils

# Problem shape (hardcoded per contract)
B, C_IN, C_OUT, K, H, W = 32, 128, 256, 3, 56, 60
HO, WO = H + K - 1, W + K - 1  # 58, 62
N_CORES = 8
BPC = B // N_CORES  # images per core
PY, PX = 62, 64  # zero-padded input plane
# Output row blocks: 7 blocks of 8 rows + 1 block of 2 rows (8*64=512 = one PSUM bank)
BLOCKS = [(y0, min(8, HO - y0)) for y0 in range(0, HO, 8)]

# DoubleRow tap pairing: (tap0, tap1) with tap=(kh,kw) or None for the zero
# tap. rhs window0 starts at row y0+kh0, col kw0; window1 is `step` elements
# later in the flat padded plane.
PAIR_TAPS = [
    ((0, 0), (1, 0)),  # step 64 (one padded row)
    ((0, 1), (1, 1)),
    ((0, 2), (1, 2)),
    ((2, 0), (2, 1)),  # step 1 (one column)
    ((2, 2), None),  # zero tap, step 64
]


def _pair_step(tap0, tap1):
    if tap1 is None:
        return PX
    return (tap1[0] - tap0[0]) * PX + (tap1[1] - tap0[1])


N_SLOTS = 2 * len(PAIR_TAPS)

_CACHE = {}
LAST_RESULT = None  # test harness introspection


def _build():
    nc = bacc.Bacc("TRN2", target_bir_lowering=False, debug=False, num_devices=N_CORES)
    fp8 = mybir.dt.float8e4
    f32 = mybir.dt.float32

    x = nc.dram_tensor("x", [BPC, C_IN, PY * PX], fp8, kind="ExternalInput").ap()
    w = nc.dram_tensor("w", [C_IN, N_SLOTS * C_OUT], fp8, kind="ExternalInput").ap()
    y = nc.dram_tensor("y", [BPC, C_OUT, HO, WO], f32, kind="ExternalOutput").ap()

    with tile.TileContext(nc) as tc:
        with ExitStack() as ctx:
            const_pool = ctx.enter_context(tc.tile_pool(name="const", bufs=1))
            psum_pool = ctx.enter_context(tc.tile_pool(name="psum", bufs=8, space="PSUM"))
            out_pool = ctx.enter_context(tc.tile_pool(name="outs", bufs=3))

            w_sb = const_pool.tile([C_IN, N_SLOTS, C_OUT], fp8, tag="w_sb")
            nc.sync.dma_start(w_sb.rearrange("p t o -> p (t o)"), w)

            # One padded-input tile per image (host pre-padded, contiguous DMA)
            in_pads = []
            for b in range(BPC):
                t = const_pool.tile([C_IN, PY * PX], fp8, tag=f"in_pad{b}")
                nc.sync.dma_start(t[:], x[b])
                in_pads.append(t)

            npairs = len(PAIR_TAPS)
            for b in range(BPC):
                for g in range(C_OUT // 128):
                    psum_ts = [
                        psum_pool.tile([128, 512], f32, tag="ps", name=f"ps_{b}_{g}_{i}")
                        for i in range(len(BLOCKS))
                    ]
                    for p, (tap0, tap1) in enumerate(PAIR_TAPS):
                        kh0, kw0 = tap0
                        step = _pair_step(tap0, tap1)
                        lhsT = w_sb[:, 2 * p : 2 * p + 2, g * 128 : (g + 1) * 128]
                        for blk, (y0, r) in enumerate(BLOCKS):
                            s = (y0 + kh0) * PX + kw0
                            base = in_pads[b]
                            rhs = bass.AP(
                                base.tensor,
                                base.offset + s,
                                [list(base.ap)[0], [step, 2], [1, r * PX]],
                            )
                            nc.tensor.matmul(
                                psum_ts[blk][:, : r * PX],
                                lhsT,
                                rhs,
                                start=(p == 0),
                                stop=(p == npairs - 1),
                                perf_mode=mybir.MatmulPerfMode.DoubleRow,
                            )
                    # Evacuate into one staging tile per (b,g); two big DMAs.
                    o = out_pool.tile([128, HO, WO], f32, tag="o")
                    for blk, (y0, r) in enumerate(BLOCKS):
                        src = psum_ts[blk].rearrange("p (y x) -> p y x", x=PX)
                        nc.vector.tensor_copy(o[:, y0 : y0 + r, :], src[:, :r, :WO])
                    half = 32  # rows 0..32 / 32..58
                    for lo, hi in ((0, half), (half, HO)):
                        nc.sync.dma_start(
                            y[b, g * 128 : (g + 1) * 128, lo:hi, :],
                            o[:, lo:hi, :],
                        )

    nc.compile()
    return nc


def kernel(inp: np.ndarray, weight: np.ndarray) -> np.ndarray:
    global LAST_RESULT
    if "nc" not in _CACHE:
        _CACHE["nc"] = _build()
    nc = _CACHE["nc"]

    dt = ml_dtypes.float8_e4m3
    inp_p = np.pad(
        np.ascontiguousarray(inp).astype(dt),
        ((0, 0), (0, 0), (2, PY - 2 - H), (2, PX - 2 - W)),
    ).reshape(B, C_IN, PY * PX)

    # weight [co, ci, kh, kw] -> [ci, slot, co] flattened
    wt = weight.transpose(2, 3, 1, 0)  # [kh, kw, ci, co]
    w_t = np.zeros((C_IN, N_SLOTS, C_OUT), dtype=dt)
    for p, (tap0, tap1) in enumerate(PAIR_TAPS):
        w_t[:, 2 * p] = wt[tap0[0], tap0[1]].astype(dt)
        if tap1 is not None:
            w_t[:, 2 * p + 1] = wt[tap1[0], tap1[1]].astype(dt)
    w_t = w_t.reshape(C_IN, N_SLOTS * C_OUT)

    in_maps = [
        {"x": inp_p[c * BPC : (c + 1) * BPC], "w": w_t} for c in range(N_CORES)
    ]
    res = bass_utils.run_bass_kernel_spmd(nc, in_maps, core_ids=list(range(N_CORES)))
    LAST_RESULT = res
    out = np.concatenate([res.results[c]["y"] for c in range(N_CORES)], axis=0)
    return out
